# revision 6
# baseline (speedup 1.0000x reference)
"""BatchHardTripletLoss (with faithful source bug) on 8 Trainium2 NeuronCores.

Reference semantics (N=8192, D=128, C=10 classes, margin=1.0):
    d(i,j)   = max(x2_i + x2_j - 2 e_i.e_j, 0)
    d_pos[i] = max_{j: same class} d(i,j)                  (includes self)
    S[i,k]   = sum_{j: class k} d(i,j);  k* = argmax_k S[i,k]
    j*       = (k*)-th negative of i in (class, index) order
    loss     = mean relu(d_pos - d(i,j*) + 1)

Key structure exploited (validated against the reference, ~1e-5 rel):
  * Only the diagonal of d clamps at 0, and the diagonal is exactly 0, so S
    has the closed form S[i,k] = cnt_k*x2_i + C_k - 2 e_i.E_k.
  * k* < 10 <= class sizes, so j* is among the first 10 members of class 0
    (anchors with label != 0) or of class 1 (anchors with label == 0).
  * d_pos only needs distances within the anchor's own class block.

Device layout: rows and columns are class-sorted; every class block is padded
to a uniform width (duplicates of the block's first member — never affect a
max; pad anchor rows are squashed via the x2a1 -inf trick). One NEFF with
static shapes serves all 8 cores; per-core variation is data-only.

Perf notes vs the first working version (30.5us):
  * All inputs ride 4 DMA doorbells on the 2 HWDGE queues (sync+scalar);
    every dma_start costs ~0.7us on its engine to enqueue AND adds a
    semaphore that the epilogue tears down at ~140ns each, so the old
    13-doorbell layout burned ~6us of startup + ~4us of teardown.
  * The [128, 2*Wp] x2_j broadcast (440KB of HBM traffic) is built on-device
    by gpsimd.partition_broadcast from a [1, 2*Wp] bf16 row (3.4KB).
  * Mining is batched: one reduce_max + two gpsimd tensor_tensor ops + one
    reduce_add over all Q tiles replaces 10 per-tile STT ops (3.1us -> 0.6us
    of engine time, mostly on the otherwise idle gpsimd).
  * Final loss math runs on gpsimd (SBUF-only ops), freeing the DVE, whose
    fused ADD_MAX_REDUCE pass over the [128, Wr] PSUM distance tiles is the
    critical path (~1.04ns/col, fixed by the DVE fp32 PSUM read rate).
"""

import numpy as np
from contextlib import ExitStack

import ml_dtypes
import concourse.bass as bass
import concourse.tile as tile
from concourse import bacc, mybir
from concourse import dve_ops
from concourse.dve_spec import Spec, Src0, Src1, C0, maxx, lower, _has_src1
from concourse.dve_uop import DveOpSpec
from concourse.bass_utils import run_bass_kernel_spmd

N_CORES = 8
C = 10
MARGIN = 1.0
P = 128
F32 = mybir.dt.float32
BF16 = mybir.dt.bfloat16
AX = mybir.AxisListType.X
ALU = mybir.AluOpType
NEG_INF = -3.0e38
PAD_NEG = -1.0e30

# stash of the last BassKernelResults (read by test.py for profiling)
last_results = None
_trace_opts: dict = {}


def _ref_add_max_reduce(in0, in1, c0, c1, c2):
    b = (np.asarray(in0, np.float32) + np.asarray(in1, np.float32))
    if isinstance(c0, np.ndarray):
        seed = np.asarray(c0, np.float32).reshape(-1, 1)
    else:
        seed = np.full((b.shape[0], 1), float(c0), np.float32)
    acc = np.maximum(seed, b.reshape(b.shape[0], -1).max(axis=-1, keepdims=True))
    return b.astype(np.float32), acc.astype(np.float32)


def _register_custom(name, spec):
    for op in dve_ops.OPS:
        if op.name == name:
            return op
    row = dve_ops._CUSTOM_DVE_ROW_BASE + len(dve_ops.OPS)
    assert row < 0x20
    dve_ops._SUB_OPCODE_FOR_NAME[name] = row
    shas = {}
    for ver in ("v3", "v4"):
        try:
            u = lower(spec, ver=ver)
            shas[ver] = DveOpSpec(name=name, opcode=row, uops=u,
                                  rd1_en=_has_src1(spec)).sha(ver)
        except Exception:
            pass
    assert shas, f"{name} failed to lower for any DVE version"
    op = dve_ops.DveOp(name, spec, subdim=False, uops_sha=shas)
    dve_ops.OPS.append(op)
    dve_ops.CUSTOM_DVE_SPECS[name] = spec
    return op


# out = in0 + in1; accum_out = max(s0, rowmax(out)).  Fuses the x2_j
# broadcast add into the hardest-positive max so each PSUM distance tile is
# consumed in a single DVE pass (native TENSOR_TENSOR_REDUCE hard-faults on
# this runtime).
ADD_MAX_REDUCE = _register_custom(
    "ADD_MAX_REDUCE_BHTL",
    Spec(body=Src0 + Src1, accum=maxx, accum_init=C0,
         reference=_ref_add_max_reduce))


def _build_program(Q: int, TB: int, Wp: int):
    """One SPMD program; all per-core variation is in the input tensors.

    Q: anchor tiles per core, TB: tiles in the main block, Wp: padded class
    window width (even).  PSUM tile per anchor tile: [win 0:Wp | aux Wp:Wp+20]
    (win chunks [0:512] and [512:Wp] stay inside one PSUM bank each, and the
    aux columns share the second bank — a matmul dst cannot cross banks).
    """
    nc = bacc.Bacc("TRN2", target_bir_lowering=False, debug=False,
                   num_devices=N_CORES)

    # big0 (sync q):   [ a01 256 | w0a 512 | a23 256 | w0b Wp-512 | a4.. ]
    # big1 (scalar q): [ sc Q*20 | w1 Wp | x2a1 as 2*Q bf16 cols ]
    # a2sc2 (sync q):  [4, Q*128 + Q*20] bf16
    # x2row (scalar q, first): [1, 2*Wp] bf16
    n_big0 = Q * P + Wp
    n_big1 = Q * 20 + Wp + 2 * Q
    big0_d = nc.dram_tensor("big0", [P, n_big0], BF16, kind="ExternalInput").ap()
    big1_d = nc.dram_tensor("big1", [P, n_big1], BF16, kind="ExternalInput").ap()
    a2_d = nc.dram_tensor("a2sc2", [4, Q * P + Q * 20], BF16,
                          kind="ExternalInput").ap()
    x2r_d = nc.dram_tensor("x2row", [1, 2 * Wp], BF16, kind="ExternalInput").ap()
    out_d = nc.dram_tensor("out", [1, 1], F32, kind="ExternalOutput").ap()

    W0A, W0B = 512, Wp - 512
    # big0 column offsets
    O_A01, O_W0A = 0, 256
    O_A23, O_W0B = 256 + W0A, 512 + W0A
    O_A4 = 512 + W0A + W0B
    # big1 column offsets
    O_SC, O_W1, O_X2A1 = 0, Q * 20, Q * 20 + Wp

    with tile.TileContext(nc) as tc, ExitStack() as ctx:
        const = ctx.enter_context(tc.tile_pool(name="const", bufs=1))
        psum = ctx.enter_context(tc.tile_pool(name="psum", bufs=3, space="PSUM"))
        psc = ctx.enter_context(tc.tile_pool(name="psc", bufs=1, space="PSUM"))
        scratch = ctx.enter_context(tc.tile_pool(name="scratch", bufs=2))

        ones_sb = const.tile([P, 1], F32)
        nc.gpsimd.memset(ones_sb[:], 1.0)
        # dummy 1x1 matmul: absorbs the PE sequencer's ~2us first-instruction
        # overhead while the input DMAs are still in flight
        psd = psc.tile([1, 1], F32, tag="pout", name="psd")
        nc.tensor.matmul(psd[:], ones_sb[:], ones_sb[:], start=True, stop=True)

        x2r_sb = const.tile([1, 2 * Wp], BF16)
        nc.scalar.dma_start(x2r_sb[:], x2r_d[:])
        big1_sb = const.tile([P, n_big1], BF16)
        nc.scalar.dma_start(big1_sb[:], big1_d[:])
        a2_sb = const.tile([4, Q * P + Q * 20], BF16)
        nc.sync.dma_start(a2_sb[:], a2_d[:])
        big0_sb = const.tile([P, n_big0], BF16)
        nc.sync.dma_start(big0_sb[:], big0_d[:])

        # on-device broadcast of the x2_j row to all 128 partitions
        x2jp = const.tile([P, 2 * Wp], BF16)
        nc.gpsimd.partition_broadcast(x2jp[:, 0:Wp], x2r_sb[:, 0:Wp])
        nc.gpsimd.partition_broadcast(x2jp[:, Wp:2 * Wp], x2r_sb[:, Wp:2 * Wp])

        mall = const.tile([P, Q], F32)         # max_j(x2_j - 2 e_i.e_j)
        sv_all = const.tile([P, Q * 20], F32)  # per-tile [S | d_cand]

        def win_lhs(t):
            if t < 2:
                return big0_sb[:, O_A01 + t * P:O_A01 + (t + 1) * P]
            if t < 4:
                return big0_sb[:, O_A23 + (t - 2) * P:O_A23 + (t - 1) * P]
            return big0_sb[:, O_A4 + (t - 4) * P:O_A4 + (t - 3) * P]

        for t in range(Q):
            blk = 0 if t < TB else 1
            lhs = win_lhs(t)
            lhs2 = a2_sb[:, t * P:(t + 1) * P]
            sc_t = big1_sb[:, O_SC + t * 20:O_SC + (t + 1) * 20]
            sc2_t = a2_sb[:, Q * P + t * 20:Q * P + (t + 1) * 20]
            if blk == 0:
                w0, w1 = big0_sb[:, O_W0A:O_W0A + W0A], \
                    big0_sb[:, O_W0B:O_W0B + W0B]
            else:
                w0, w1 = big1_sb[:, O_W1:O_W1 + W0A], \
                    big1_sb[:, O_W1 + W0A:O_W1 + Wp]

            ps = psum.tile([P, Wp + 20], F32, tag="ps", name=f"ps{t}")
            nc.tensor.matmul(ps[:, Wp:Wp + 20], lhs, sc_t, start=True, stop=False)
            nc.tensor.matmul(ps[:, 0:W0A], lhs, w0, start=True, stop=True)
            nc.tensor.matmul(ps[:, W0A:Wp], lhs, w1, start=True, stop=True)
            nc.tensor.matmul(ps[:, Wp:Wp + 20], lhs2, sc2_t, start=False,
                             stop=True)

            dsc = scratch.tile([P, Wp], F32)
            nc.vector._custom_dve(ADD_MAX_REDUCE, out=dsc[:],
                                  in0=ps[:, 0:Wp],
                                  in1=x2jp[:, blk * Wp:(blk + 1) * Wp],
                                  s0=NEG_INF, accum_out=mall[:, t:t + 1])
            nc.scalar.copy(sv_all[:, t * 20:(t + 1) * 20], ps[:, Wp:Wp + 20])

        # ---- batched mining epilogue ----
        sv3 = sv_all[:].rearrange("p (q s) -> p q s", s=20)
        smax = const.tile([P, Q], F32)
        nc.vector.reduce_max(smax[:], sv3[:, :, 0:10], axis=AX)
        eq = const.tile([P, Q * 10], F32)
        eq3 = eq[:].rearrange("p (q s) -> p q s", s=10)
        smax_b, _ = bass.broadcast_tensor_aps(smax[:].unsqueeze(2), eq3)
        nc.vector.tensor_tensor(eq3, sv3[:, :, 0:10], smax_b, op=ALU.is_equal)
        pr = const.tile([P, Q * 10], F32)
        pr3 = pr[:].rearrange("p (q s) -> p q s", s=10)
        nc.gpsimd.tensor_tensor(pr3, eq3, sv3[:, :, 10:20], op=ALU.mult)
        dneg = const.tile([P, Q], F32)
        nc.vector.reduce_sum(dneg[:], pr3, axis=AX)

        # loss = relu(mall + (x2_i | -inf pad) - dneg + margin), summed
        x2a1 = big1_sb[:, O_X2A1:O_X2A1 + 2 * Q].bitcast(F32)
        t1 = const.tile([P, Q], F32)
        nc.gpsimd.tensor_tensor(t1[:], mall[:], x2a1, op=ALU.add)
        t2 = const.tile([P, Q], F32)
        nc.gpsimd.tensor_tensor(t2[:], t1[:], dneg[:], op=ALU.subtract)
        t3 = const.tile([P, Q], F32)
        nc.vector.tensor_scalar(t3[:], t2[:], MARGIN, 0.0,
                                op0=ALU.add, op1=ALU.max)  # relu(x + margin)
        lsum = const.tile([P, 1], F32)
        nc.vector.reduce_sum(lsum[:], t3[:], axis=AX)
        # partition-sum via a 1-column matmul so the output DMA is a single
        # 4-byte transfer
        pout = psc.tile([1, 1], F32, tag="pout")
        nc.tensor.matmul(pout[:], lsum[:], ones_sb[:], start=True, stop=True)
        res_sb = const.tile([1, 1], F32)
        nc.scalar.copy(res_sb[:], pout[:])
        nc.sync.dma_start(out_d[:], res_sb[:])

    nc.compile()
    return nc


_prog_cache: dict = {}


def kernel(embeddings: np.ndarray, labels: np.ndarray) -> np.ndarray:
    global last_results
    e = np.ascontiguousarray(np.asarray(embeddings), dtype=np.float32)
    lab = np.asarray(labels).astype(np.int64)
    N, D = e.shape
    assert D == P and N % N_CORES == 0

    # ---- host-side marshalling: class-sort, pad, per-class stats ----
    order = np.argsort(lab * N + np.arange(N))
    e = e[order]
    lab_s = lab[order]
    cnt = np.bincount(lab_s, minlength=C)
    assert len(cnt) == C and cnt[0] >= 10 and cnt[1] >= 10, cnt
    offs = np.zeros(C + 1, dtype=np.int64)
    offs[1:] = np.cumsum(cnt)

    # block width: multiple of 512 with C*B/128 tiles splitting evenly
    # across 8 cores -> B in {1024, 1536, ...}
    B = 1024
    while cnt.max() > B or (C * (B // P)) % N_CORES != 0:
        B += 512
    TB = B // P
    Q = C * TB // N_CORES
    L = Q - TB  # leftover tiles per core

    x2 = np.einsum("nd,nd->n", e, e).astype(np.float32)
    NP_ = C * B
    ep = np.empty((NP_, D), np.float32)
    x2p = np.empty(NP_, np.float32)
    validp = np.zeros(NP_, np.float32)
    for k in range(C):
        m = int(cnt[k])
        blk = e[offs[k]:offs[k + 1]]
        ep[k * B:k * B + m] = blk
        ep[k * B + m:(k + 1) * B] = blk[0]
        x2p[k * B:k * B + m] = x2[offs[k]:offs[k + 1]]
        x2p[k * B + m:(k + 1) * B] = x2[offs[k]]
        validp[k * B:k * B + m] = 1.0
    # bf16-rounded x2_i, shared by the fp32 aux matmul and x2a1 so the x2_i
    # term cancels exactly in d_pos - d_neg
    x2p_bf32 = x2p.astype(ml_dtypes.bfloat16).astype(np.float32)

    E = np.stack([e[offs[k]:offs[k + 1]].sum(axis=0) for k in range(C)],
                 axis=1).astype(np.float32)          # [D, C]
    Ck = np.array([x2[offs[k]:offs[k + 1]].sum() for k in range(C)],
                  dtype=np.float32)                  # [C]
    candA = e[0:10]                                  # class-0 members
    candB = e[offs[1]:offs[1] + 10]                  # class-1 members
    x2A, x2B = x2[0:10], x2[offs[1]:offs[1] + 10]

    Wr = int(cnt.max())
    Wp = Wr + (Wr & 1)
    assert Wp >= 514 and Wp <= B
    key = (Q, TB, Wp)
    if key not in _prog_cache:
        _prog_cache[key] = _build_program(Q, TB, Wp)
    nc = _prog_cache[key]

    W0A = 512
    in_maps = []
    for c in range(N_CORES):
        mb = c                        # main block
        eb = N_CORES + (c * L) // TB  # leftover block index
        et = (c * L) % TB             # first leftover tile within it
        rows = np.concatenate([
            np.arange(mb * B, (mb + 1) * B),
            np.arange(eb * B + et * P, eb * B + (et + L) * P),
        ])
        tile_cls = [mb] * TB + [eb] * L
        wcols = np.concatenate([np.arange(mb * B, mb * B + Wp),
                                np.arange(eb * B, eb * B + Wp)])

        anchT = ep[rows].T                          # [D, Q*128]
        a = (-2.0 * anchT).astype(ml_dtypes.bfloat16)
        # aux lhsT rows [x2_i; 1; x2_i; 1] pair with hi/lo-split rhs rows so
        # every aux product is bf16-exact (bf16 alone cannot hold cnt_k / C_k)
        a2 = np.stack([x2p_bf32[rows], np.ones(Q * P, np.float32),
                       x2p_bf32[rows], np.ones(Q * P, np.float32)])
        w = ep[wcols].T.astype(ml_dtypes.bfloat16)   # [D, 2*Wp]
        x2row = x2p[wcols][None, :].astype(ml_dtypes.bfloat16)
        sc = np.empty((D, Q * 20), np.float32)
        sc2 = np.empty((4, Q * 20), np.float32)
        cnt_f = cnt.astype(np.float32)
        cnt_hi = (cnt // 128 * 128).astype(np.float32)
        cnt_lo = cnt_f - cnt_hi
        Ck_hi = Ck.astype(ml_dtypes.bfloat16).astype(np.float32)
        Ck_lo = Ck - Ck_hi
        x2A_hi = x2A.astype(ml_dtypes.bfloat16).astype(np.float32)
        x2B_hi = x2B.astype(ml_dtypes.bfloat16).astype(np.float32)
        for t in range(Q):
            c0 = tile_cls[t] == 0
            cand = candB if c0 else candA
            x2c_hi = x2B_hi if c0 else x2A_hi
            x2c_lo = (x2B - x2B_hi) if c0 else (x2A - x2A_hi)
            sc[:, t * 20:t * 20 + 10] = E
            sc[:, t * 20 + 10:t * 20 + 20] = cand.T
            sc2[0, t * 20:t * 20 + 10] = cnt_hi
            sc2[1, t * 20:t * 20 + 10] = Ck_hi
            sc2[2, t * 20:t * 20 + 10] = cnt_lo
            sc2[3, t * 20:t * 20 + 10] = Ck_lo
            sc2[0, t * 20 + 10:t * 20 + 20] = 1.0
            sc2[1, t * 20 + 10:t * 20 + 20] = x2c_hi
            sc2[2, t * 20 + 10:t * 20 + 20] = 0.0
            sc2[3, t * 20 + 10:t * 20 + 20] = x2c_lo
        vmask = validp[rows].reshape(Q, P).T
        x2a1 = np.where(vmask > 0.5,
                        x2p_bf32[rows].reshape(Q, P).T,
                        PAD_NEG).astype(np.float32)

        ab = a  # [128, Q*128] bf16
        wb = w  # [128, 2*Wp]
        big0 = np.concatenate([
            ab[:, 0:2 * P],                # a01
            wb[:, 0:W0A],                  # w0a
            ab[:, 2 * P:4 * P],            # a23
            wb[:, W0A:Wp],                 # w0b
            ab[:, 4 * P:Q * P],            # a4..
        ], axis=1)
        big1 = np.concatenate([
            sc.astype(ml_dtypes.bfloat16),
            wb[:, Wp:2 * Wp],
            np.ascontiguousarray(x2a1).view(ml_dtypes.bfloat16),
        ], axis=1)
        a2sc2 = np.concatenate([a2.astype(ml_dtypes.bfloat16),
                                sc2.astype(ml_dtypes.bfloat16)], axis=1)

        in_maps.append({"big0": big0, "big1": big1, "a2sc2": a2sc2,
                        "x2row": x2row})

    res = run_bass_kernel_spmd(nc, in_maps, list(range(N_CORES)), **_trace_opts)
    last_results = res
    total = np.float64(0.0)
    for c in range(N_CORES):
        total += res.results[c]["out"].astype(np.float64).sum()
    return np.asarray(total / N, dtype=np.float32)
